# revision 1
# baseline (speedup 1.0000x reference)
"""Trainium2 Bass kernel for a tanh-RNN (nn_RNN_39084202393635).

    embedded = emb[input]                      # [B,T,H] gather
    ix_t     = embedded @ Wi + bi              # hoisted, parallel
    h_t      = tanh(ix_t + h_{t-1} @ Wh + bh)  # sequential scan over T
    out_t    = h_t @ Wo + bo                   # hoisted out of the loop, parallel

Sharding: data-parallel over batch, 8 rows per core, weights/emb replicated.
All on-chip state is kept H-major ([H, batch] "transposed" layout) so the
sequential scan needs no per-step transposes: each step is 64 self-loading
matmuls (Wh 128x128 bf16 tiles stationary, h streaming) accumulating into 8
PSUM banks, then a DVE add of ix and a fused tanh+bias on the ACT engine
emitting the next bf16 state directly.

Everything (weights, gathered+transposed embeddings, ix, all hidden states)
lives in SBUF; DRAM traffic is just the input gather and the output store.
"""

import os
import sys

_TRN_REPO = "/opt/trn_rl_repo"
if os.path.isdir(_TRN_REPO) and _TRN_REPO not in sys.path:
    sys.path.insert(0, _TRN_REPO)

from contextlib import ExitStack

import ml_dtypes
import numpy as np

import concourse.bass as bass
import concourse.tile as tile
from concourse import bacc, bass_utils, mybir
from concourse.masks import make_identity

BF16 = ml_dtypes.bfloat16

V, H, O = 50257, 1024, 1024
B, T = 64, 256
NCORES = 8
BL = B // NCORES          # batch rows per core
NT = T * BL               # 2048 (t, b) columns, t-major
P = 128
KC = H // P               # contraction chunks
MC = H // P               # output-H chunks
GT = NT // P              # gather tiles of 128 rows
NI = 512                  # ix/outproj matmul free-dim chunk
SH = T + 1                # hidden-state slots (incl. h0)

f32 = mybir.dt.float32
bf16 = mybir.dt.bfloat16
i32 = mybir.dt.int32


def emit(nc):
    ids_g = nc.dram_tensor("ids_g", [GT, P], i32, kind="ExternalInput").ap()
    emb_b = nc.dram_tensor("emb_b", [V, H], bf16, kind="ExternalInput").ap()
    wi_b = nc.dram_tensor("wi_b", [H, H], bf16, kind="ExternalInput").ap()
    wh_b = nc.dram_tensor("wh_b", [H, H], bf16, kind="ExternalInput").ap()
    wo_b = nc.dram_tensor("wo_b", [H, H], bf16, kind="ExternalInput").ap()
    h0t = nc.dram_tensor("h0t", [H, BL], bf16, kind="ExternalInput").ap()
    bib = nc.dram_tensor("bib", [P, MC], f32, kind="ExternalInput").ap()
    bo_r = nc.dram_tensor("bo_r", [P, H], f32, kind="ExternalInput").ap()
    out_l = nc.dram_tensor("out_l", [NT, O], f32, kind="ExternalOutput").ap()
    ht_o = nc.dram_tensor("ht_o", [H, BL], bf16, kind="ExternalOutput").ap()

    with tile.TileContext(nc) as tc, ExitStack() as ctx:
        const = ctx.enter_context(tc.tile_pool(name="const", bufs=1))

        WI = const.tile([P, KC * H], bf16, tag="wi")
        WH = const.tile([P, KC * H], bf16, tag="wh")
        WO = const.tile([P, KC * H], bf16, tag="wo")
        EMBT = const.tile([P, KC * NT], bf16, tag="embt")
        IXT = const.tile([P, MC * NT], bf16, tag="ixt")
        HALL = const.tile([P, KC * SH * BL], bf16, tag="hall")
        BIB = const.tile([P, MC], f32, tag="bib")
        BO = const.tile([P, H], f32, tag="bo")
        IDN = const.tile([P, P], bf16, tag="idn")

        for k in range(KC):
            nc.sync.dma_start(WI[:, k * H:(k + 1) * H], wi_b[k * P:(k + 1) * P, :])
            nc.sync.dma_start(WH[:, k * H:(k + 1) * H], wh_b[k * P:(k + 1) * P, :])
            nc.sync.dma_start(WO[:, k * H:(k + 1) * H], wo_b[k * P:(k + 1) * P, :])
            hb = k * SH * BL
            nc.sync.dma_start(HALL[:, hb:hb + BL], h0t[k * P:(k + 1) * P, :])
        nc.sync.dma_start(BIB[:], bib[:])
        nc.sync.dma_start(BO[:], bo_r[:])
        make_identity(nc, IDN[:])

        # Phase A: gather embedding rows, transpose to H-major, project by Wi.
        with ExitStack() as pctx:
            idxp = pctx.enter_context(tc.tile_pool(name="idx", bufs=3))
            stg = pctx.enter_context(tc.tile_pool(name="stg", bufs=3))
            tpp = pctx.enter_context(tc.tile_pool(name="tp", bufs=2, space="PSUM"))
            psx = pctx.enter_context(tc.tile_pool(name="psx", bufs=4, space="PSUM"))
            for n in range(NT // NI):
                for gg in range(NI // P):
                    g = n * (NI // P) + gg
                    idx = idxp.tile([P, 1], i32)
                    nc.sync.dma_start(idx[:], ids_g[g:g + 1, :])
                    st = stg.tile([P, H], bf16)
                    nc.gpsimd.indirect_dma_start(
                        st[:], None, emb_b,
                        bass.IndirectOffsetOnAxis(ap=idx[:, :1], axis=0),
                    )
                    for k in range(KC):
                        tp = tpp.tile([P, P], bf16)
                        nc.tensor.transpose(tp[:], st[:, k * P:(k + 1) * P], IDN[:])
                        nc.vector.tensor_copy(
                            EMBT[:, k * NT + g * P: k * NT + (g + 1) * P], tp[:])
                for m in range(MC):
                    ps = psx.tile([P, NI], f32)
                    for k in range(KC):
                        nc.tensor.matmul(
                            ps[:],
                            lhsT=WI[:, k * H + m * P: k * H + (m + 1) * P],
                            rhs=EMBT[:, k * NT + n * NI: k * NT + (n + 1) * NI],
                            start=(k == 0), stop=(k == KC - 1),
                        )
                    nc.vector.tensor_copy(
                        IXT[:, m * NT + n * NI: m * NT + (n + 1) * NI], ps[:])

        # Phase B: the sequential scan.
        with ExitStack() as pctx:
            pss = pctx.enter_context(tc.tile_pool(name="pss", bufs=8, space="PSUM"))
            tmpp = pctx.enter_context(tc.tile_pool(name="tmp", bufs=4))
            for t in range(T):
                s0 = t * BL
                s1 = (t + 1) * BL
                for m in range(MC):
                    ps = pss.tile([P, BL], f32)
                    for k in range(KC):
                        nc.tensor.matmul(
                            ps[:],
                            lhsT=WH[:, k * H + m * P: k * H + (m + 1) * P],
                            rhs=HALL[:, k * SH * BL + s0: k * SH * BL + s0 + BL],
                            start=(k == 0), stop=(k == KC - 1),
                        )
                    tm = tmpp.tile([P, BL], f32)
                    nc.vector.tensor_tensor(
                        tm[:], ps[:], IXT[:, m * NT + s0: m * NT + s0 + BL],
                        op=mybir.AluOpType.add)
                    nc.scalar.activation(
                        HALL[:, m * SH * BL + s1: m * SH * BL + s1 + BL], tm[:],
                        mybir.ActivationFunctionType.Tanh, bias=BIB[:, m:m + 1])

        # Phase C: output projection out = h @ Wo + bo, rows (t,b) t-major.
        with ExitStack() as pctx:
            pso = pctx.enter_context(tc.tile_pool(name="pso", bufs=4, space="PSUM"))
            outp = pctx.enter_context(tc.tile_pool(name="outp", bufs=4))
            for mc in range(NT // P):
                c0 = (mc * (P // BL) + 1) * BL  # state slots t+1 .. t+16
                for n in range(O // NI):
                    ps = pso.tile([P, NI], f32)
                    for k in range(KC):
                        nc.tensor.matmul(
                            ps[:],
                            lhsT=HALL[:, k * SH * BL + c0: k * SH * BL + c0 + P],
                            rhs=WO[:, k * H + n * NI: k * H + (n + 1) * NI],
                            start=(k == 0), stop=(k == KC - 1),
                        )
                    o = outp.tile([P, NI], f32)
                    nc.vector.tensor_tensor(
                        o[:], ps[:], BO[:, n * NI:(n + 1) * NI],
                        op=mybir.AluOpType.add)
                    nc.sync.dma_start(
                        out_l[mc * P:(mc + 1) * P, n * NI:(n + 1) * NI], o[:])
            for k in range(KC):
                nc.sync.dma_start(
                    ht_o[k * P:(k + 1) * P, :],
                    HALL[:, k * SH * BL + T * BL: k * SH * BL + (T + 1) * BL])


_COMPILED = None


def get_compiled():
    global _COMPILED
    if _COMPILED is None:
        nc = bacc.Bacc("TRN2", target_bir_lowering=False, debug=False)
        emit(nc)
        nc.compile()
        _COMPILED = nc
    return _COMPILED


def make_in_maps(input, hidden, emb, Wi, bi, Wh, bh, Wo, bo):
    input = np.asarray(input)
    hidden = np.asarray(hidden, np.float32)
    emb_b = np.asarray(emb, np.float32).astype(BF16)
    wi_b = np.asarray(Wi, np.float32).astype(BF16)
    wh_b = np.asarray(Wh, np.float32).astype(BF16)
    wo_b = np.asarray(Wo, np.float32).astype(BF16)
    bib = np.ascontiguousarray(
        (np.asarray(bi, np.float32) + np.asarray(bh, np.float32)).reshape(MC, P).T)
    bo_r = np.ascontiguousarray(
        np.tile(np.asarray(bo, np.float32)[None, :], (P, 1)))
    in_maps = []
    for c in range(NCORES):
        rows = slice(c * BL, (c + 1) * BL)
        idsg = np.ascontiguousarray(
            np.asarray(input[rows], np.int32).T).reshape(GT, P)
        h0t = np.ascontiguousarray(hidden[rows].T).astype(BF16)
        in_maps.append({
            "ids_g": idsg, "emb_b": emb_b, "wi_b": wi_b, "wh_b": wh_b,
            "wo_b": wo_b, "h0t": h0t, "bib": bib, "bo_r": bo_r,
        })
    return in_maps


def assemble(results):
    outs = []
    hids = []
    for c in range(NCORES):
        ol = np.asarray(results[c]["out_l"], np.float32)
        outs.append(ol.reshape(T, BL, O).transpose(1, 0, 2))
        hids.append(np.asarray(results[c]["ht_o"], np.float32).T)
    return (np.ascontiguousarray(np.concatenate(outs, 0)),
            np.ascontiguousarray(np.concatenate(hids, 0)))


def kernel(input, hidden, emb, Wi, bi, Wh, bh, Wo, bo):
    nc = get_compiled()
    in_maps = make_in_maps(input, hidden, emb, Wi, bi, Wh, bh, Wo, bo)
    res = bass_utils.run_bass_kernel_spmd(nc, in_maps, core_ids=list(range(NCORES)))
    return assemble(res.results)


# revision 12
# speedup vs baseline: 1.1312x; 1.1312x over previous
"""Trainium2 Bass kernel for a tanh-RNN (nn_RNN_39084202393635).

    embedded = emb[input]                      # [B,T,H] gather
    ix_t     = embedded @ Wi + bi              # hoisted, parallel
    h_t      = tanh(ix_t + h_{t-1} @ Wh + bh)  # sequential scan over T
    out_t    = h_t @ Wo + bo                   # hoisted out of the loop, parallel

Sharding: data-parallel over batch, 8 rows per core, weights/emb replicated.

All on-chip state is kept H-major ([H, batch] "transposed" layout) so the
sequential scan needs no per-step transposes: each step is 64 self-loading
matmuls (Wh 128x128 bf16 tiles stationary, h streaming) accumulating into two
PSUM groups, a DVE add of the precomputed ix (biases folded in), and a tanh on
the ACT engine emitting the next bf16 state directly.

The scan is weight-load bound on the PE (every step must re-feed all of Wh
into the array), so the embedding gather / transpose / ix projection and the
output projection are interleaved item-by-item into the scan's 256 steps,
where their DMA and streaming cost hides in the weight-load shadow. Only the
first 16 time-steps' worth of ix is computed up front.
"""

import os
import sys

_TRN_REPO = "/opt/trn_rl_repo"
if os.path.isdir(_TRN_REPO) and _TRN_REPO not in sys.path:
    sys.path.insert(0, _TRN_REPO)

from contextlib import ExitStack

import ml_dtypes
import numpy as np

import concourse.bass as bass
import concourse.tile as tile
from concourse import bacc, bass_utils, mybir
from concourse.masks import make_identity

BF16 = ml_dtypes.bfloat16

V, H, O = 50257, 1024, 1024
B, T = 64, 256
NCORES = 8
BL = B // NCORES          # batch rows per core
NT = T * BL               # 2048 (t, b) columns, t-major
P = 128
KC = H // P               # contraction chunks
MC = H // P               # output-H chunks
GT = NT // P              # gather tiles of 128 (t,b) rows; 1 tile = 16 steps
NI = 512                  # outproj matmul free-dim chunk
SH = T + 1                # hidden-state slots (incl. h0)
SG = 2                    # scan PSUM groups per step
GM = MC // SG             # m-chunks per scan group

f32 = mybir.dt.float32
bf16 = mybir.dt.bfloat16
i32 = mybir.dt.int32

TANH = mybir.ActivationFunctionType.Tanh
ADD = mybir.AluOpType.add


def emit(nc, rep=1):
    ids_g = nc.dram_tensor("ids_g", [P, GT], i32, kind="ExternalInput").ap()
    emb_b = nc.dram_tensor("emb_b", [V, H], bf16, kind="ExternalInput").ap()
    wi_b = nc.dram_tensor("wi_b", [H, H], bf16, kind="ExternalInput").ap()
    wh_b = nc.dram_tensor("wh_b", [H, H], bf16, kind="ExternalInput").ap()
    wo_b = nc.dram_tensor("wo_b", [H, H], bf16, kind="ExternalInput").ap()
    h0t = nc.dram_tensor("h0t", [H, BL], bf16, kind="ExternalInput").ap()
    bib = nc.dram_tensor("bib", [P, MC], f32, kind="ExternalInput").ap()
    bo_r = nc.dram_tensor("bo_r", [P, H], f32, kind="ExternalInput").ap()
    out_l = nc.dram_tensor("out_l", [NT, O], f32, kind="ExternalOutput").ap()
    ht_o = nc.dram_tensor("ht_o", [H, BL], bf16, kind="ExternalOutput").ap()

    with tile.TileContext(nc) as tc, ExitStack() as ctx:
        const = ctx.enter_context(tc.tile_pool(name="const", bufs=1))

        WI = const.tile([P, KC * H], bf16, tag="wi")
        WH = const.tile([P, KC * H], bf16, tag="wh")
        WO = const.tile([P, KC * H], bf16, tag="wo")
        IXT = const.tile([P, MC * NT], bf16, tag="ixt")   # [H_out, (t,b)] m-major
        HALL = const.tile([P, KC, SH * BL], bf16, tag="hall")
        BIB = const.tile([P, MC], f32, tag="bib")
        BO = const.tile([P, H], f32, tag="bo")
        IDN = const.tile([P, P], bf16, tag="idn")
        IDXS = const.tile([P, GT], i32, tag="idxs")

        for k in range(KC):
            nc.sync.dma_start(WI[:, k * H:(k + 1) * H], wi_b[k * P:(k + 1) * P, :])
            nc.sync.dma_start(WH[:, k * H:(k + 1) * H], wh_b[k * P:(k + 1) * P, :])
            nc.sync.dma_start(WO[:, k * H:(k + 1) * H], wo_b[k * P:(k + 1) * P, :])
            nc.sync.dma_start(HALL[:, k:k + 1, 0:BL], h0t[k * P:(k + 1) * P, :])
        nc.sync.dma_start(BIB[:], bib[:])
        nc.sync.dma_start(BO[:], bo_r[:])
        nc.sync.dma_start(IDXS[:], ids_g[:])
        make_identity(nc, IDN[:])

        with ExitStack() as pctx:
            stg = pctx.enter_context(tc.tile_pool(name="stg", bufs=3))
            embt = pctx.enter_context(tc.tile_pool(name="embt", bufs=3))
            tpp = pctx.enter_context(tc.tile_pool(name="tp", bufs=2, space="PSUM"))
            psx = pctx.enter_context(tc.tile_pool(name="psx", bufs=2, space="PSUM"))
            pss = pctx.enter_context(tc.tile_pool(name="pss", bufs=3, space="PSUM"))
            pso = pctx.enter_context(tc.tile_pool(name="pso", bufs=1, space="PSUM"))
            tmpp = pctx.enter_context(tc.tile_pool(name="tmp", bufs=4))
            outp = pctx.enter_context(tc.tile_pool(name="outp", bufs=2))

            def a_items(g):
                """Work items for gather tile g: gather 128 embedding rows,
                transpose to H-major, project through Wi into IXT cols g*P.."""
                def gather():
                    st = stg.tile([P, H], bf16, tag="st", name=f"st{g}")
                    nc.gpsimd.indirect_dma_start(
                        st[:], None, emb_b,
                        bass.IndirectOffsetOnAxis(ap=IDXS[:, g:g + 1], axis=0))
                    return st

                state = {}

                def do_gather():
                    state["st"] = gather()
                    state["et"] = embt.tile([P, KC * P], bf16, tag="et", name=f"et{g}")
                yield do_gather

                def transpose_k(k):
                    def run():
                        tp = tpp.tile([P, P], bf16, tag="tp", name=f"tp{g}_{k}")
                        nc.tensor.transpose(
                            tp[:], state["st"][:, k * P:(k + 1) * P], IDN[:])
                        nc.vector.tensor_copy(
                            state["et"][:, k * P:(k + 1) * P], tp[:])
                    return run
                for k in range(KC):
                    yield transpose_k(k)

                def mm_m(m):
                    def run():
                        ps = psx.tile([P, P], f32, tag="psx", name=f"psx{g}_{m}")
                        for k in range(KC):
                            nc.tensor.matmul(
                                ps[:],
                                lhsT=WI[:, k * H + m * P: k * H + (m + 1) * P],
                                rhs=state["et"][:, k * P:(k + 1) * P],
                                start=(k == 0), stop=(k == KC - 1))
                        nc.vector.tensor_tensor(
                            IXT[:, m * NT + g * P: m * NT + (g + 1) * P], ps[:],
                            BIB[:, m:m + 1].to_broadcast([P, P]), op=ADD)
                    return run
                for m in range(MC):
                    yield mm_m(m)

            def c_items(mc):
                """Work items for output rows mc*P..: project 16 steps of h."""
                c0 = (mc * (P // BL) + 1) * BL  # state slots t+1 .. t+16
                state = {}

                def alloc():
                    state["o"] = outp.tile([P, O], f32, tag="o", name=f"o{mc}")
                yield alloc

                def mm(n, k):
                    def run():
                        if k == 0:
                            state["ps"] = pso.tile([P, NI], f32, tag="pso", name=f"pso{mc}_{n}")
                        nc.tensor.matmul(
                            state["ps"][:],
                            lhsT=HALL[:, k:k + 1, c0:c0 + P],
                            rhs=WO[:, k * H + n * NI: k * H + (n + 1) * NI],
                            start=(k == 0), stop=(k == KC - 1))
                        if k == KC - 1:
                            nc.vector.tensor_tensor(
                                state["o"][:, n * NI:(n + 1) * NI], state["ps"][:],
                                BO[:, n * NI:(n + 1) * NI], op=ADD)
                    return run
                for n in range(O // NI):
                    for k in range(KC):
                        yield mm(n, k)

                def store():
                    nc.sync.dma_start(out_l[mc * P:(mc + 1) * P, :], state["o"][:])
                yield store

            IXT3 = IXT[:].rearrange("p (m tb) -> p m tb", m=MC)

            def scan_step(t):
                s0 = t * BL
                s1 = (t + 1) * BL
                for g2 in range(SG):
                    ps = pss.tile([P, GM * BL], f32, tag="pss", name=f"pss{t}_{g2}")
                    for k in range(KC):
                        for mi in range(GM):
                            m = g2 * GM + mi
                            # start clears the whole bank's has_written bits, so
                            # it must fire only on the tile's first matmul; the
                            # per-element bits make later first-writes overwrite.
                            nc.tensor.matmul(
                                ps[:, mi * BL:(mi + 1) * BL],
                                lhsT=WH[:, k * H + m * P: k * H + (m + 1) * P],
                                rhs=HALL[:, k:k + 1, s0:s0 + BL],
                                start=(k == 0 and mi == 0),
                                stop=(k == KC - 1 and mi == GM - 1),
                                skip_group_check=True)
                    tm = tmpp.tile([P, GM * BL], f32, tag="tm", name=f"tm{t}_{g2}")
                    nc.vector.tensor_tensor(
                        tm[:].rearrange("p (c b) -> p c b", b=BL), ps[:].rearrange(
                            "p (c b) -> p c b", b=BL),
                        IXT3[:, g2 * GM:(g2 + 1) * GM, s0:s0 + BL], op=ADD)
                    nc.scalar.activation(
                        HALL[:, g2 * GM:(g2 + 1) * GM, s1:s1 + BL],
                        tm[:].rearrange("p (c b) -> p c b", b=BL), TANH)

            def body(chain):
                if chain:
                    # timing builds only: serialize reps via h0 <- h_final
                    for k in range(KC):
                        nc.vector.tensor_copy(
                            HALL[:, k:k + 1, 0:BL],
                            HALL[:, k:k + 1, T * BL:(T + 1) * BL])

                # schedule: slots[t] = list of interleave closures
                slots = [[] for _ in range(T + 1)]
                for g in range(GT):
                    items = list(a_items(g))
                    if g == 0:
                        slots[0][:0] = items  # prologue, before step 0
                    else:
                        w0 = (g - 1) * 16    # window start; deadline step 16g
                        for i, it in enumerate(items):
                            slots[min(w0 + (i * 15) // len(items), 16 * g - 1)
                                  ].append(it)
                for mc in range(GT):
                    items = list(c_items(mc))
                    w0 = mc * 16 + 17
                    for i, it in enumerate(items):
                        slots[min(w0 + (i * 15) // len(items), T)].append(it)

                for it in slots[0]:
                    it()
                for t in range(T):
                    scan_step(t)
                    for it in slots[t + 1]:
                        it()

            for r in range(rep):
                body(chain=(r > 0))

            for k in range(KC):
                nc.sync.dma_start(ht_o[k * P:(k + 1) * P, :],
                                  HALL[:, k:k + 1, T * BL:(T + 1) * BL])


_COMPILED = None


def get_compiled():
    global _COMPILED
    if _COMPILED is None:
        nc = bacc.Bacc("TRN2", target_bir_lowering=False, debug=False)
        emit(nc)
        nc.compile()
        _COMPILED = nc
    return _COMPILED


def make_in_maps(input, hidden, emb, Wi, bi, Wh, bh, Wo, bo):
    input = np.asarray(input)
    hidden = np.asarray(hidden, np.float32)
    emb_b = np.asarray(emb, np.float32).astype(BF16)
    wi_b = np.asarray(Wi, np.float32).astype(BF16)
    wh_b = np.asarray(Wh, np.float32).astype(BF16)
    wo_b = np.asarray(Wo, np.float32).astype(BF16)
    bib = np.ascontiguousarray(
        (np.asarray(bi, np.float32) + np.asarray(bh, np.float32)).reshape(MC, P).T)
    bo_r = np.ascontiguousarray(
        np.tile(np.asarray(bo, np.float32)[None, :], (P, 1)))
    in_maps = []
    for c in range(NCORES):
        rows = slice(c * BL, (c + 1) * BL)
        idsg = np.ascontiguousarray(
            np.asarray(input[rows], np.int32).T.reshape(GT, P).T)
        h0t = np.ascontiguousarray(hidden[rows].T).astype(BF16)
        in_maps.append({
            "ids_g": idsg, "emb_b": emb_b, "wi_b": wi_b, "wh_b": wh_b,
            "wo_b": wo_b, "h0t": h0t, "bib": bib, "bo_r": bo_r,
        })
    return in_maps


def assemble(results):
    outs = []
    hids = []
    for c in range(NCORES):
        ol = np.asarray(results[c]["out_l"], np.float32)
        outs.append(ol.reshape(T, BL, O).transpose(1, 0, 2))
        hids.append(np.asarray(results[c]["ht_o"], np.float32).T)
    return (np.ascontiguousarray(np.concatenate(outs, 0)),
            np.ascontiguousarray(np.concatenate(hids, 0)))


def kernel(input, hidden, emb, Wi, bi, Wh, bh, Wo, bo):
    nc = get_compiled()
    in_maps = make_in_maps(input, hidden, emb, Wi, bi, Wh, bh, Wo, bo)
    res = bass_utils.run_bass_kernel_spmd(nc, in_maps, core_ids=list(range(NCORES)))
    return assemble(res.results)
